# revision 14
# baseline (speedup 1.0000x reference)
"""Category-specific linear: out[b] = x[b] @ weight[cat[b]] + bias[cat[b]].

Full shapes: x [32, 512, 1024] f32, category_ids [32] int, weight
[64, 1024, 1024] f32, bias [64, 1024] f32 -> out [32, 512, 1024] f32.

Strategy: data-parallel over batch across 8 NeuronCores (4 batches/core).
All device-side numerics run in fp16: the host gathers per-batch weights,
pre-transposes x to [K, L], and casts both to fp16 (halving the HBM
stream vs f32); the device writes fp16 output and the host casts up,
restores the layout, and adds the bias. With a 16 MB per-core HBM stream
the kernel is PE-bound (~55 us of matmul at the 2.4 GHz fast clock), so
everything serves the matmul issue rate:

- x is the stationary operand (lhsT = xt[k, lt] tile [128K x 128L]) and
  w the moving one (rhs = w[k] in two [128K x 512N] chunks), so each
  stationary tile serves 2 consecutive matmuls and the PE array's
  weight-swap drain is paid once per pair. PSUM holds 8 bank tiles
  [128L x 512N] per batch; measured issue rate 216 ns/matmul.
- A short run of dummy matmuls on garbage SBUF warms the PE HAM clock
  gate during the framework preamble + first-chunk DMA fill, so real
  matmuls start at the fast clock.
- Inputs stream as 32 uniform chunks (batch, k-tile) in consumption
  order on the sync/SP HWDGE ring. The host packs xt|w for each chunk
  contiguously (p-major, 3 KB per-partition lines), so a chunk is ONE
  384 KB DMA: issue cost (~0.6 us each) stays far below the PE's 1.7
  us/chunk consumption rate, and per-chunk semaphores let the PE trail
  the stream by a single chunk.
- PSUM eviction of each row tile is split column-wise between the DVE
  (vector) and ACT (scalar) engines - two parallel copy-with-cast ops -
  so the next batch's matmuls never stall on bank reuse. Outputs leave
  as two 512 KB DMAs per batch on the scalar/ACT ring, which carries no
  input traffic.
"""

from contextlib import ExitStack

import numpy as np

import concourse.bass as bass
import concourse.mybir as mybir
from concourse.bass_utils import run_bass_kernel_spmd

# Per-core problem shape
B = 4           # batches per core
L = 512         # rows (seq positions) per batch
K = 1024        # contraction dim
N = 1024        # output dim
KT = K // 128   # 8 k-tiles
LT = L // 128   # 4 l-tiles (psum row tiles per batch, 2 banks each)
NWARM = 12      # dummy matmuls to warm the PE clock before inputs land

F32 = mybir.dt.float32
F16 = mybir.dt.float16
NP_DT = np.float16

CH = L + N       # 1536: packed chunk width (xt k-tile | w k-tile)
BBUF = KT * CH   # 12288 fp16 per partition per batch
OBUF = LT * N    # 4096


def build_program() -> bass.Bass:
    nc = bass.Bass()

    # p-major packing: element [b, p, k*CH + f] is k-row k*128+p,
    # f in [0,512) -> xt col f;  f in [512,1536) -> w col f-512
    in_d = nc.declare_dram_parameter("inp", [B, 128, BBUF], F16, isOutput=False)
    out_d = nc.declare_dram_parameter("out", [B, 128, OBUF], F16, isOutput=True)

    with ExitStack() as ctx:
        # all 4 batches resident: 96 KB/part inputs + 16 KB out
        in_sb = ctx.enter_context(nc.sbuf_tensor([128, B * BBUF], F16))
        out_sb = ctx.enter_context(nc.sbuf_tensor([128, 2 * OBUF], F16))
        psum = ctx.enter_context(nc.psum_tensor([128, 8 * 512], F32))  # 8 banks
        s_chunk = [ctx.enter_context(nc.semaphore(f"s_c{k}")) for k in range(KT)]
        s_o = [ctx.enter_context(nc.semaphore(f"s_o{b}")) for b in range(B)]
        s_mm = ctx.enter_context(nc.semaphore("s_mm"))
        s_cpv = ctx.enter_context(nc.semaphore("s_cpv"))
        s_cps = ctx.enter_context(nc.semaphore("s_cps"))
        block = ctx.enter_context(nc.Block())

        def xt_tile(b, k, lt):
            base = b * BBUF + k * CH + lt * 128
            return in_sb[:, base : base + 128]

        def w_half(b, k, nh):
            base = b * BBUF + k * CH + L + nh * 512
            return in_sb[:, base : base + 512]

        def load_chunk(eng, b, k, lo, hi):
            eng.dma_start(
                out=in_sb[:, b * BBUF + k * CH + lo : b * BBUF + k * CH + hi],
                in_=in_d[b, :, k * CH + lo : k * CH + hi],
            ).then_inc(s_chunk[k], 16)

        @block.sync
        def _(sync):
            # chunk (0,0) split: xt + w-half0 first so matmuls start sooner
            load_chunk(sync, 0, 0, 0, L + 512)
            load_chunk(sync, 0, 0, L + 512, CH)
            for k in range(2, KT, 2):
                load_chunk(sync, 0, k, 0, CH)
            for b in range(1, B):
                for k in range(KT):
                    load_chunk(sync, b, k, 0, CH)
            # tail assist: left half (row tile 2) of batch 3's last out chunk
            sync.wait_ge(s_cpv, 3 * LT + 3)
            sync.wait_ge(s_cps, 3 * LT + 3)
            sync.dma_start(
                out=out_d[B - 1, :, 2 * N : 3 * N],
                in_=out_sb[:, (B - 1) % 2 * OBUF + 2 * N : (B - 1) % 2 * OBUF + 3 * N],
            ).then_inc(s_o[B - 1], 16)
            for b in range(B - 1):
                sync.wait_ge(s_o[b], 32)
            sync.wait_ge(s_o[B - 1], 48)
            sync.drain()

        @block.scalar
        def _(scalar):
            # batch 0 odd k-chunks ride the ACT ring, parallel with sync's
            for k in range(1, KT, 2):
                load_chunk(scalar, 0, k, 0, CH)
            # evictions (n-half 1 of each row tile) + output DMAs
            for b in range(B):
                obuf = b % 2
                if b >= 2:
                    scalar.wait_ge(s_o[b - 2], 32)
                for lt in range(LT):
                    scalar.wait_ge(s_mm, b * 2 * LT + lt * 2 + 2)
                    nc.scalar.copy(
                        out=out_sb[
                            :,
                            obuf * OBUF + lt * N + 512 : obuf * OBUF + (lt + 1) * N,
                        ],
                        in_=psum[:, (lt * 2 + 1) * 512 : (lt * 2 + 2) * 512],
                    ).then_inc(s_cps, 1)
                    if lt % 2 == 1:
                        # out chunk c = row tiles lt-1, lt; the last chunk's
                        # left half goes out on the sync ring (tail assist)
                        c = lt // 2
                        lo = c * 2 * N
                        hi = (c + 1) * 2 * N
                        if b == B - 1 and c == 1:
                            lo += N
                        scalar.wait_ge(s_cpv, b * LT + lt + 1)
                        scalar.dma_start(
                            out=out_d[b, :, lo:hi],
                            in_=out_sb[:, obuf * OBUF + lo : obuf * OBUF + hi],
                        ).then_inc(s_o[b], 16)

        @block.tensor
        def _(tensor):
            # warm the HAM clock gate while the preamble + first DMA run
            for i in range(NWARM):
                nc.tensor.matmul(
                    psum[:, 0:512],
                    in_sb[:, 0:128],
                    in_sb[:, L : L + 512],
                    start=True,
                    stop=True,
                )
            def mm_at(b, k, lt, nh):
                t = lt * 2 + nh
                mm = nc.tensor.matmul(
                    psum[:, t * 512 : (t + 1) * 512],
                    xt_tile(b, k, lt),
                    w_half(b, k, nh),
                    start=(k == 0),
                    stop=(k == KT - 1),
                )
                if k == KT - 1:
                    mm.then_inc(s_mm, 1)

            # batch 0, k=0: nh-grouped so the nh=0 matmuls only wait for the
            # first (xt + w-half0) slice of the split chunk
            tensor.wait_ge(s_chunk[0], 16)
            for lt in range(LT):
                mm_at(0, 0, lt, 0)
            tensor.wait_ge(s_chunk[0], 32)
            for lt in range(LT):
                mm_at(0, 0, lt, 1)
            for b in range(B):
                for k in range(KT):
                    if b == 0 and k == 0:
                        continue
                    # chunk (0,0) was two DMAs, so k=0 counts run 16 high
                    tensor.wait_ge(s_chunk[k], 16 * (b + 1) + (16 if k == 0 else 0))
                    for lt in range(LT):
                        for nh in range(2):
                            if k == 0 and b > 0:
                                # tile must have been evicted from batch b-1
                                sem = s_cpv if nh == 0 else s_cps
                                tensor.wait_ge(sem, (b - 1) * LT + lt + 1)
                            mm_at(b, k, lt, nh)

        @block.vector
        def _(vector):
            # evictions: n-half 0 of every (lt) row tile
            for b in range(B):
                obuf = b % 2
                if b >= 2:
                    vector.wait_ge(s_o[b - 2], 32)
                for lt in range(LT):
                    vector.wait_ge(s_mm, b * 2 * LT + lt * 2 + 1)
                    nc.vector.tensor_copy(
                        out=out_sb[:, obuf * OBUF + lt * N : obuf * OBUF + lt * N + 512],
                        in_=psum[:, lt * 2 * 512 : (lt * 2 + 1) * 512],
                    ).then_inc(s_cpv, 1)

    return nc


_NC = None


def _get_program():
    global _NC
    if _NC is None:
        _NC = build_program()
    return _NC


def make_in_maps(x, category_ids, weight, bias=None):
    x = np.asarray(x, dtype=np.float32)
    cids = np.asarray(category_ids).astype(np.int64)
    weight = np.asarray(weight, dtype=np.float32)

    # xt: [32, K, L] -> p-major per-k [32, 128, KT, L]
    xt = np.ascontiguousarray(x.transpose(0, 2, 1)).astype(NP_DT)
    xt = xt.reshape(32, KT, 128, L).transpose(0, 2, 1, 3)
    # w: [32, K, N] -> p-major per-k [32, 128, KT, N]
    wg = weight[cids].astype(NP_DT)
    wg = wg.reshape(32, KT, 128, N).transpose(0, 2, 1, 3)
    # pack [xt_k | w_k] chunks: [32, 128, KT, CH] -> [32, 128, BBUF]
    packed = np.concatenate([xt, wg], axis=3).reshape(32, 128, BBUF)

    in_maps = []
    for c in range(8):
        sl = slice(c * B, (c + 1) * B)
        in_maps.append({"inp": np.ascontiguousarray(packed[sl])})
    return in_maps


def run_on_device(in_maps, **kwargs):
    return run_bass_kernel_spmd(_get_program(), in_maps, list(range(8)), **kwargs)


def kernel(x, category_ids, weight, bias=None):
    in_maps = make_in_maps(x, category_ids, weight)
    res = run_on_device(in_maps)
    out = np.concatenate([res.results[c]["out"] for c in range(8)], axis=0)
    # [32, 128, LT*N] p-major -> [32, L, N]
    out = out.astype(np.float32).reshape(32, 128, LT, N).transpose(0, 2, 1, 3)
    out = out.reshape(32, L, N)
    cids = np.asarray(category_ids).astype(np.int64)
    if bias is None:
        bias = np.zeros((np.asarray(weight).shape[0], N), dtype=np.float32)
    out = out + np.asarray(bias, dtype=np.float32)[cids][:, None, :]
    return np.ascontiguousarray(out.astype(np.float32))


# revision 15
# speedup vs baseline: 1.0508x; 1.0508x over previous
"""Category-specific linear: out[b] = x[b] @ weight[cat[b]] + bias[cat[b]].

Full shapes: x [32, 512, 1024] f32, category_ids [32] int, weight
[64, 1024, 1024] f32, bias [64, 1024] f32 -> out [32, 512, 1024] f32.

Strategy: data-parallel over batch across 8 NeuronCores (4 batches/core).
All device-side numerics run in fp16: the host gathers per-batch weights,
pre-transposes x to [K, L], and casts both to fp16 (halving the HBM
stream vs f32); the device writes fp16 output and the host casts up,
restores the layout, and adds the bias. With a 16 MB per-core HBM stream
the kernel is PE-bound (~55 us of matmul at the 2.4 GHz fast clock), so
everything serves the matmul issue rate:

- x is the stationary operand (lhsT = xt[k, lt] tile [128K x 128L]) and
  w the moving one (rhs = w[k] in two [128K x 512N] chunks), so each
  stationary tile serves 2 consecutive matmuls and the PE array's
  weight-swap drain is paid once per pair. PSUM holds 8 bank tiles
  [128L x 512N] per batch; measured issue rate 216 ns/matmul.
- A short run of dummy matmuls on garbage SBUF warms the PE HAM clock
  gate during the framework preamble + first-chunk DMA fill, so real
  matmuls start at the fast clock.
- Inputs stream as 32 uniform chunks (batch, k-tile) in consumption
  order on the sync/SP HWDGE ring. The host packs xt|w for each chunk
  contiguously (p-major, 3 KB per-partition lines), so a chunk is ONE
  384 KB DMA: issue cost (~0.6 us each) stays far below the PE's 1.7
  us/chunk consumption rate, and per-chunk semaphores let the PE trail
  the stream by a single chunk.
- PSUM eviction of each row tile is split column-wise between the DVE
  (vector) and ACT (scalar) engines - two parallel copy-with-cast ops -
  so the next batch's matmuls never stall on bank reuse. Outputs leave
  as two 512 KB DMAs per batch on the scalar/ACT ring, which carries no
  input traffic.
"""

from contextlib import ExitStack

import numpy as np

import concourse.bass as bass
import concourse.mybir as mybir
from concourse.bass_utils import run_bass_kernel_spmd

# Per-core problem shape
B = 4           # batches per core
L = 512         # rows (seq positions) per batch
K = 1024        # contraction dim
N = 1024        # output dim
KT = K // 128   # 8 k-tiles
LT = L // 128   # 4 l-tiles (psum row tiles per batch, 2 banks each)
NWARM = 12      # dummy matmuls to warm the PE clock before inputs land

F32 = mybir.dt.float32
F16 = mybir.dt.float16
NP_DT = np.float16

CH = L + N       # 1536: packed chunk width (xt k-tile | w k-tile)
BBUF = KT * CH   # 12288 fp16 per partition per batch
OBUF = LT * N    # 4096


def build_program() -> bass.Bass:
    nc = bass.Bass()

    # p-major packing: element [b, p, k*CH + f] is k-row k*128+p,
    # f in [0,512) -> xt col f;  f in [512,1536) -> w col f-512
    in_d = nc.declare_dram_parameter("inp", [B, 128, BBUF], F16, isOutput=False)
    out_d = nc.declare_dram_parameter("out", [B, 128, OBUF], F16, isOutput=True)

    with ExitStack() as ctx:
        # all 4 batches resident: 96 KB/part inputs + 16 KB out
        in_sb = ctx.enter_context(nc.sbuf_tensor([128, B * BBUF], F16))
        out_sb = ctx.enter_context(nc.sbuf_tensor([128, 2 * OBUF], F16))
        psum = ctx.enter_context(nc.psum_tensor([128, 8 * 512], F32))  # 8 banks
        s_chunk = [ctx.enter_context(nc.semaphore(f"s_c{k}")) for k in range(KT)]
        s_o = [ctx.enter_context(nc.semaphore(f"s_o{b}")) for b in range(B)]
        s_mm = ctx.enter_context(nc.semaphore("s_mm"))
        s_cpv = ctx.enter_context(nc.semaphore("s_cpv"))
        s_cps = ctx.enter_context(nc.semaphore("s_cps"))
        block = ctx.enter_context(nc.Block())

        def xt_tile(b, k, lt):
            base = b * BBUF + k * CH + lt * 128
            return in_sb[:, base : base + 128]

        def w_half(b, k, nh):
            base = b * BBUF + k * CH + L + nh * 512
            return in_sb[:, base : base + 512]

        def load_chunk(eng, b, k, lo, hi):
            eng.dma_start(
                out=in_sb[:, b * BBUF + k * CH + lo : b * BBUF + k * CH + hi],
                in_=in_d[b, :, k * CH + lo : k * CH + hi],
            ).then_inc(s_chunk[k], 16)

        @block.sync
        def _(sync):
            # chunk (0,0) split: xt + w-half0 first so matmuls start sooner
            load_chunk(sync, 0, 0, 0, L + 512)
            load_chunk(sync, 0, 0, L + 512, CH)
            for k in range(2, KT, 2):
                load_chunk(sync, 0, k, 0, CH)
            for b in range(1, B):
                for k in range(KT):
                    load_chunk(sync, b, k, 0, CH)
            # tail assist: left half (row tile 2) of batch 3's last out chunk
            sync.wait_ge(s_cpv, 3 * LT + 3)
            sync.wait_ge(s_cps, 3 * LT + 3)
            sync.dma_start(
                out=out_d[B - 1, :, 2 * N : 3 * N],
                in_=out_sb[:, (B - 1) % 2 * OBUF + 2 * N : (B - 1) % 2 * OBUF + 3 * N],
            ).then_inc(s_o[B - 1], 16)
            for b in range(B - 1):
                sync.wait_ge(s_o[b], 32)
            sync.wait_ge(s_o[B - 1], 48)
            sync.drain()

        @block.scalar
        def _(scalar):
            # batch 0 odd k-chunks ride the ACT ring, parallel with sync's
            for k in range(1, KT, 2):
                load_chunk(scalar, 0, k, 0, CH)
            # evictions (n-half 1 of each row tile) + output DMAs
            for b in range(B):
                obuf = b % 2
                if b >= 2:
                    scalar.wait_ge(s_o[b - 2], 32)
                for lt in range(LT):
                    scalar.wait_ge(s_mm, b * 2 * LT + lt * 2 + 2)
                    nc.scalar.copy(
                        out=out_sb[
                            :,
                            obuf * OBUF + lt * N + 512 : obuf * OBUF + (lt + 1) * N,
                        ],
                        in_=psum[:, (lt * 2 + 1) * 512 : (lt * 2 + 2) * 512],
                    ).then_inc(s_cps, 1)
                    if lt % 2 == 1:
                        # out chunk c = row tiles lt-1, lt; the last chunk's
                        # left half goes out on the sync ring (tail assist)
                        c = lt // 2
                        lo = c * 2 * N
                        hi = (c + 1) * 2 * N
                        if b == B - 1 and c == 1:
                            lo += N
                        scalar.wait_ge(s_cpv, b * LT + lt + 1)
                        scalar.dma_start(
                            out=out_d[b, :, lo:hi],
                            in_=out_sb[:, obuf * OBUF + lo : obuf * OBUF + hi],
                        ).then_inc(s_o[b], 16)

        @block.tensor
        def _(tensor):
            # warm the HAM clock gate while the preamble + first DMA run
            for i in range(NWARM):
                nc.tensor.matmul(
                    psum[:, 0:512],
                    in_sb[:, 0:128],
                    in_sb[:, L : L + 512],
                    start=True,
                    stop=True,
                )
            def mm_at(b, k, lt, nh):
                t = lt * 2 + nh
                mm = nc.tensor.matmul(
                    psum[:, t * 512 : (t + 1) * 512],
                    xt_tile(b, k, lt),
                    w_half(b, k, nh),
                    start=(k == 0),
                    stop=(k == KT - 1),
                )
                if k == KT - 1:
                    mm.then_inc(s_mm, 1)

            # batch 0, k=0: nh-grouped so the nh=0 matmuls only wait for the
            # first (xt + w-half0) slice of the split chunk
            tensor.wait_ge(s_chunk[0], 16)
            for lt in range(LT):
                mm_at(0, 0, lt, 0)
            tensor.wait_ge(s_chunk[0], 32)
            for lt in range(LT):
                mm_at(0, 0, lt, 1)
            for b in range(B):
                for k in range(KT - 2):
                    if b == 0 and k == 0:
                        continue
                    # chunk (0,0) was two DMAs, so k=0 counts run 16 high
                    tensor.wait_ge(s_chunk[k], 16 * (b + 1) + (16 if k == 0 else 0))
                    for lt in range(LT):
                        for nh in range(2):
                            if k == 0 and b > 0:
                                # tile must have been evicted from batch b-1
                                sem = s_cpv if nh == 0 else s_cps
                                tensor.wait_ge(sem, (b - 1) * LT + lt + 1)
                            mm_at(b, k, lt, nh)
                # last two k-rounds interleave per tile, so each tile's final
                # matmul (and its eviction) lands well before the batch ends
                tensor.wait_ge(s_chunk[KT - 2], 16 * (b + 1))
                tensor.wait_ge(s_chunk[KT - 1], 16 * (b + 1))
                for lt in range(LT):
                    for k in (KT - 2, KT - 1):
                        for nh in range(2):
                            mm_at(b, k, lt, nh)

        @block.vector
        def _(vector):
            # evictions: n-half 0 of every (lt) row tile
            for b in range(B):
                obuf = b % 2
                if b >= 2:
                    vector.wait_ge(s_o[b - 2], 32)
                for lt in range(LT):
                    vector.wait_ge(s_mm, b * 2 * LT + lt * 2 + 1)
                    nc.vector.tensor_copy(
                        out=out_sb[:, obuf * OBUF + lt * N : obuf * OBUF + lt * N + 512],
                        in_=psum[:, lt * 2 * 512 : (lt * 2 + 1) * 512],
                    ).then_inc(s_cpv, 1)

    return nc


_NC = None


def _get_program():
    global _NC
    if _NC is None:
        _NC = build_program()
    return _NC


def make_in_maps(x, category_ids, weight, bias=None):
    x = np.asarray(x, dtype=np.float32)
    cids = np.asarray(category_ids).astype(np.int64)
    weight = np.asarray(weight, dtype=np.float32)

    # xt: [32, K, L] -> p-major per-k [32, 128, KT, L]
    xt = np.ascontiguousarray(x.transpose(0, 2, 1)).astype(NP_DT)
    xt = xt.reshape(32, KT, 128, L).transpose(0, 2, 1, 3)
    # w: [32, K, N] -> p-major per-k [32, 128, KT, N]
    wg = weight[cids].astype(NP_DT)
    wg = wg.reshape(32, KT, 128, N).transpose(0, 2, 1, 3)
    # pack [xt_k | w_k] chunks: [32, 128, KT, CH] -> [32, 128, BBUF]
    packed = np.concatenate([xt, wg], axis=3).reshape(32, 128, BBUF)

    in_maps = []
    for c in range(8):
        sl = slice(c * B, (c + 1) * B)
        in_maps.append({"inp": np.ascontiguousarray(packed[sl])})
    return in_maps


def run_on_device(in_maps, **kwargs):
    return run_bass_kernel_spmd(_get_program(), in_maps, list(range(8)), **kwargs)


def kernel(x, category_ids, weight, bias=None):
    in_maps = make_in_maps(x, category_ids, weight)
    res = run_on_device(in_maps)
    out = np.concatenate([res.results[c]["out"] for c in range(8)], axis=0)
    # [32, 128, LT*N] p-major -> [32, L, N]
    out = out.astype(np.float32).reshape(32, 128, LT, N).transpose(0, 2, 1, 3)
    out = out.reshape(32, L, N)
    cids = np.asarray(category_ids).astype(np.int64)
    if bias is None:
        bias = np.zeros((np.asarray(weight).shape[0], N), dtype=np.float32)
    out = out + np.asarray(bias, dtype=np.float32)[cids][:, None, :]
    return np.ascontiguousarray(out.astype(np.float32))
